# revision 42
# baseline (speedup 1.0000x reference)
"""Causal self-attention (B=4, T=2048, C=1024, 16 heads) on 8 trn2 NeuronCores.

Sharding: core (b, hg) handles batch b (4) x head-group hg (2 groups of 8 heads).
Each core computes QKV projection for its batch restricted to its 8 heads,
flash-style causal attention, and the output projection restricted to its
heads' rows of w_proj -> a partial [T, C] output. Host sums the two partials
per batch (tensor-parallel unshard) and concatenates batches.

v2 layout (all bf16 matmul inputs, fp32 PSUM accumulation):
  - q-chunk-outer loop (c = 512 queries), head-pair inner. Per (c, pair, slot):
    scores pair (two K=64 row-tiled matmuls run concurrently on the PE),
    ScalarE exp (one instruction per slot; diagonal slots combine both heads
    via a strided AP), then the A@V matmuls for that slot accumulate into the
    pair's two PSUM tiles -- a 3-stage PE/ScalarE pipeline with 2-deep PSUM
    pools and a 2-slot score lookahead.
  - V carries an appended ones-column so A@V (lhsT=[V|1], N=512) yields
    y~^T = [64 weighted-V | row-sum].  The [65,512] result is copied to bf16
    and PE-transposed in 128-q blocks into one [P,4,65] PSUM tile; a single
    batched reciprocal + 4 tensor_scalar ops produce normalized y.
  - y [t,ch] is PE-transposed to y^T per q-chunk and the output projection +
    output DMA run per chunk, keeping the PE warm (no transpose-only tail that
    would let the HAM clock gate re-throttle) and spreading the output DMA.
  - The next chunk's QKV projection matmuls are injected into the attention
    slot loops as PE filler while ScalarE works through the exps; the last
    chunk's K/Q projections are injected just-in-time one head-pair ahead.
  - Input DMAs split across the two HWDGE queues (Sync: x^T + w_v; Scalar:
    everything else) with a fine-grained first chunk so the first V-proj
    matmul issues as early as possible.
"""

import numpy as np
import ml_dtypes

B, T, C, H, D = 4, 2048, 1024, 16, 64
P = 128
TC = T // P          # 16 t-chunks of 128
KC = C // P          # 8 contraction chunks of 128
NPAIR = 4            # head pairs per core (8 local heads)
SCALE = 0.125        # 1/sqrt(64)

_CACHE = {}
LAST_RESULT = None   # BassKernelResults of the most recent run (for test.py)

BF16 = ml_dtypes.bfloat16


def _build_program():
    import concourse.tile as tile
    import concourse.mybir as mybir
    from concourse import bacc

    dt = mybir.dt
    AF = mybir.ActivationFunctionType
    ALU = mybir.AluOpType

    nc = bacc.Bacc("TRN2", target_bir_lowering=False, debug=False, num_devices=8)

    # ---- DRAM I/O ----
    xT_d = nc.dram_tensor("xT", [C, T], dt.bfloat16, kind="ExternalInput").ap()
    # wqk pre-permuted on host: [m, p, k*128] (m = 8 output 128-col blocks,
    # Q blocks 0-3 then K blocks 4-7; p = partition; k = contraction chunk)
    wqk_d = nc.dram_tensor("wqk", [8, P, 1024], dt.bfloat16, kind="ExternalInput").ap()
    wv_d = nc.dram_tensor("wv", [C, 512], dt.bfloat16, kind="ExternalInput").ap()
    wproj_d = nc.dram_tensor("wproj", [512, C], dt.bfloat16, kind="ExternalInput").ap()
    bqk_d = nc.dram_tensor("bqk", [P, 8], dt.float32, kind="ExternalInput").ap()
    bv_d = nc.dram_tensor("bv", [P, 512], dt.float32, kind="ExternalInput").ap()
    bproj_d = nc.dram_tensor("bproj", [P, C], dt.float32, kind="ExternalInput").ap()
    dmask_d = nc.dram_tensor("dmask", [P, P], dt.bfloat16, kind="ExternalInput").ap()
    ident_d = nc.dram_tensor("ident", [P, P], dt.bfloat16, kind="ExternalInput").ap()
    out_d = nc.dram_tensor("out", [T, C], dt.bfloat16, kind="ExternalOutput").ap()

    with tile.TileContext(nc) as tc:
        with (
            tc.tile_pool(name="const", bufs=1) as cp,
            tc.tile_pool(name="outp", bufs=4) as op_pool,
            tc.tile_pool(name="small", bufs=8) as sp,
            tc.tile_pool(name="ytmp", bufs=3) as yt_pool,
            tc.tile_pool(name="psS", bufs=2, space="PSUM") as psS_pool,
            tc.tile_pool(name="psav", bufs=2, space="PSUM") as av_pool,
            tc.tile_pool(name="psshr", bufs=2, space="PSUM") as shr_pool,
        ):
            # ---- static SBUF tensors ----
            xT_s = cp.tile([P, KC, T], dt.bfloat16, name="xT_s")
            wqk_s = cp.tile([P, 8, KC, 128], dt.bfloat16, name="wqk_s")
            wv_s = cp.tile([P, KC, 512], dt.bfloat16, name="wv_s")
            wproj_s = cp.tile([P, 4, C], dt.bfloat16, name="wproj_s")
            bqk_s = cp.tile([P, 8], dt.float32, name="bqk_s")
            bv_s = cp.tile([P, 512], dt.float32, name="bv_s")
            bproj_s = cp.tile([P, C], dt.float32, name="bproj_s")
            dmask_s = cp.tile([P, P], dt.bfloat16, name="dmask_s")
            ident_s = cp.tile([P, P], dt.bfloat16, name="ident_s")
            qt_s = cp.tile([P, NPAIR, T], dt.bfloat16, name="qt_s")
            kt_s = cp.tile([P, NPAIR, T], dt.bfloat16, name="kt_s")
            v_s = cp.tile([P, TC, 8, 66], dt.bfloat16, name="v_s")   # [t, tc, head, V|1]
            y_s = cp.tile([P, TC, 8, D], dt.bfloat16, name="y_s")    # y natural [q, head, d]
            yT_s = cp.tile([P, 2, 4, 512], dt.bfloat16, name="yT_s")  # y^T per chunk (x2)
            pt_s = cp.tile([P, 12, 2, 512], dt.bfloat16, name="pt_s")   # exp(S^T) off-diag
            ptd_s = cp.tile([P, 4, 2, 512], dt.bfloat16, name="ptd_s")  # diag slots

            warm_s = cp.tile([P, 512], dt.bfloat16, name="warm_s")
            # ones column of V~; zero the diag P^T buffer once (sub-diagonal
            # regions are never written by the partial exps, so zeros persist)
            nc.vector.memset(warm_s[:], 0.0)
            nc.vector.memset(v_s[:, :, :, 64:65], 1.0)
            nc.vector.memset(ptd_s[:], 0.0)

            # ---- input DMAs split across the two HWDGE queues ----
            xT_src = xT_d.rearrange("(o p) t -> p o t", p=P)
            wv_src = wv_d.rearrange("(o p) m -> p o m", p=P)
            # Sync queue: the big tensors, in exact consumption order (fine
            # first chunks so the first V-proj matmul can issue early; wqk
            # split by head-pair column ranges to match qk emission order)
            nc.sync.dma_start(wv_s[:, 0:1, :], wv_src[:, 0:1, :])
            nc.sync.dma_start(xT_s[:, :, 0:128], xT_src[:, :, 0:128])
            nc.sync.dma_start(wv_s[:, 1:2, :], wv_src[:, 1:2, :])
            nc.sync.dma_start(wv_s[:, 2:4, :], wv_src[:, 2:4, :])
            nc.sync.dma_start(wv_s[:, 4:6, :], wv_src[:, 4:6, :])
            nc.sync.dma_start(wv_s[:, 6:8, :], wv_src[:, 6:8, :])
            nc.sync.dma_start(xT_s[:, :, 128:256], xT_src[:, :, 128:256])
            nc.sync.dma_start(xT_s[:, :, 256:512], xT_src[:, :, 256:512])
            for p_ in range(4):
                nc.sync.dma_start(wqk_s[:, p_, :, :], wqk_d[p_])
                nc.sync.dma_start(wqk_s[:, 4 + p_, :, :], wqk_d[4 + p_])
            for q8 in range(2, 8):
                nc.sync.dma_start(xT_s[:, :, 256 * q8:256 * (q8 + 1)],
                                  xT_src[:, :, 256 * q8:256 * (q8 + 1)])
            # Scalar queue: small/late tensors (prologue-only; exps start later)
            nc.scalar.dma_start(bv_s[:], bv_d)
            nc.scalar.dma_start(bqk_s[:], bqk_d)
            nc.scalar.dma_start(dmask_s[:], dmask_d)
            nc.scalar.dma_start(ident_s[:], ident_d)
            nc.scalar.dma_start(wproj_s[:], wproj_d.rearrange("(o p) m -> p o m", p=P))
            nc.scalar.dma_start(bproj_s[:], bproj_d)

            # ---- projection chunk emitters (prologue + slot-loop fillers) ----
            def vproj_chunk(tcx):
                psv = shr_pool.tile([P, 512], dt.float32, name="psv", tag="shr")
                for k in range(KC):
                    nc.tensor.matmul(psv[:, :],
                                     xT_s[:, k, P * tcx:P * (tcx + 1)],
                                     wv_s[:, k, :],
                                     start=(k == 0), stop=(k == KC - 1))
                nc.vector.tensor_add(
                    out=v_s[:, tcx, :, 0:64],
                    in0=psv[:, :].rearrange("a (h d) -> a h d", h=8),
                    in1=bv_s[:, :].rearrange("a (h d) -> a h d", h=8),
                )

            def qkproj_chunk(m, t4):
                dst = qt_s if m < 4 else kt_s
                psq = shr_pool.tile([P, 512], dt.float32, name="psq", tag="shr")
                for k in range(KC):
                    nc.tensor.matmul(psq[:, :],
                                     wqk_s[:, m, k, :],
                                     xT_s[:, k, 512 * t4:512 * (t4 + 1)],
                                     start=(k == 0), stop=(k == KC - 1))
                nc.vector.tensor_scalar(
                    out=dst[:, m % 4, 512 * t4:512 * (t4 + 1)],
                    in0=psq[:, :], scalar1=bqk_s[:, m:m + 1], scalar2=None,
                    op0=ALU.add)

            from collections import deque
            fill_q = deque()

            def pop_filler(n=1):
                for _ in range(n):
                    if fill_q:
                        fill_q.popleft()()

            def stage2_chunk(c, tcx_loc, cc):
                """Transpose one [128q, 128ch] block of y into y^T for proj."""
                tcx = 4 * c + tcx_loc
                pstr = shr_pool.tile([P, P], dt.bfloat16, name="pstr", tag="shr")
                nc.tensor.transpose(pstr[:, :],
                                    y_s[:, tcx, 2 * cc:2 * cc + 2, :],
                                    ident_s[:, :])
                nc.vector.tensor_copy(
                    out=yT_s[:, c % 2, cc, P * tcx_loc:P * (tcx_loc + 1)],
                    in_=pstr[:, :])

            # ---- HAM warmup: dummy matmuls on the zeroed tile bridge the
            # clock-gate activity window across the head's DMA stalls so the
            # PE-dense QK-projection phase runs at full clock ----
            pswarm = shr_pool.tile([P, 512], dt.float32, name="pswarm", tag="shr")
            for wi in range(12):
                nc.tensor.matmul(pswarm[:, :], warm_s[:, 0:128], warm_s[:, :],
                                 start=(wi == 0), stop=(wi == 11))

            # ---- c=0 prologue (emitted directly). First 4 t-chunks run
            # k-outer so the PE consumes each wv k-chunk as it lands instead
            # of stalling on the full wv upfront ----
            psv0 = [shr_pool.tile([P, 512], dt.float32, name=f"psv0_{t}",
                                  tag="shr") for t in range(2)]
            psv1 = [psS_pool.tile([P, 512], dt.float32, name=f"psv1_{t}",
                                  tag="psS") for t in range(2)]
            pv = psv0 + psv1
            for k in range(KC):
                for tcx in range(4):
                    nc.tensor.matmul(pv[tcx][:, :],
                                     xT_s[:, k, P * tcx:P * (tcx + 1)],
                                     wv_s[:, k, :],
                                     start=(k == 0), stop=(k == KC - 1))
            for tcx in range(4):
                nc.vector.tensor_add(
                    out=v_s[:, tcx, :, 0:64],
                    in0=pv[tcx][:, :].rearrange("a (h d) -> a h d", h=8),
                    in1=bv_s[:, :].rearrange("a (h d) -> a h d", h=8),
                )
            for m in (0, 4, 1, 5, 2, 6, 3, 7):
                qkproj_chunk(m, 0)

            # ---- helper: scores pair for (c, pair, slot j) ----
            def scores_slot(c, pair, j):
                r = j - 4 * c
                q0 = P * r if r >= 0 else 0
                psS = psS_pool.tile([P, 1024], dt.float32, name="psS", tag="psS")
                for hh in (0, 1):
                    base = 64 * hh
                    nc.tensor.matmul(
                        psS[:, 512 * hh + q0:512 * (hh + 1)],
                        kt_s[base:base + 64, pair, P * j:P * (j + 1)],
                        qt_s[base:base + 64, pair, 512 * c + q0:512 * (c + 1)],
                        start=True, stop=True)
                return psS

            def emit_tail(c):
                """Chunk c's y -> y^T transposes and output projection."""
                cb = c % 2
                for tcx_loc in range(4):
                    for cc in range(4):
                        stage2_chunk(c, tcx_loc, cc)
                    pop_filler(1)
                for tcx_loc in range(4):
                    tcx = 4 * c + tcx_loc
                    for co in range(2):
                        psp = psS_pool.tile([P, 512], dt.float32, name="psp",
                                            tag="psS")
                        for cc in range(4):
                            nc.tensor.matmul(
                                psp[:, :],
                                yT_s[:, cb, cc, P * tcx_loc:P * (tcx_loc + 1)],
                                wproj_s[:, cc, 512 * co:512 * (co + 1)],
                                start=(cc == 0), stop=(cc == 3))
                        ot = op_pool.tile([P, 512], dt.bfloat16, name="ot",
                                          tag="ot")
                        nc.vector.tensor_add(out=ot[:, :], in0=psp[:, :],
                                             in1=bproj_s[:, 512 * co:512 * (co + 1)])
                        nc.sync.dma_start(
                            out_d[P * tcx:P * (tcx + 1), 512 * co:512 * (co + 1)],
                            ot[:, :])
                    pop_filler(1)

            # ---- main attention loop: q-chunk outer, head-pair inner ----
            for c in range(4):
                nj = 4 * c + 4
                # stock the filler queue with next chunk's projections
                if c == 0:
                    for tcx in range(4, 8):
                        fill_q.append(lambda tcx=tcx: vproj_chunk(tcx))
                    for m in (0, 4, 1, 5, 2, 6, 3, 7):
                        fill_q.append(lambda m=m: qkproj_chunk(m, 1))
                elif c == 1:
                    for tcx in range(8, 12):
                        fill_q.append(lambda tcx=tcx: vproj_chunk(tcx))
                    for m in (0, 4, 1, 5, 2, 6, 3, 7):
                        fill_q.append(lambda m=m: qkproj_chunk(m, 2))
                elif c == 2:
                    for tcx in range(12, 16):
                        fill_q.append(lambda tcx=tcx: vproj_chunk(tcx))
                    for m in (0, 4):
                        fill_q.append(lambda m=m: qkproj_chunk(m, 3))

                for pair in range(NPAIR):
                    if c == 3 and pair < 3:
                        # just-in-time Q/K projections for the next head pair
                        for m in (pair + 1, 5 + pair):
                            fill_q.append(lambda m=m: qkproj_chunk(m, 3))


                    psyt = [av_pool.tile([P, 512], dt.float32, name=f"psyt{hh}",
                                         tag="av") for hh in (0, 1)]
                    slotS = [scores_slot(c, pair, 0), scores_slot(c, pair, 1)
                             if nj > 1 else None]
                    for j in range(nj):
                        r = j - 4 * c
                        q0 = P * r if r >= 0 else 0
                        psS = slotS[j % 2]
                        # exp( S^T * scale ), fp32 psum -> bf16 sbuf
                        if r < 0:
                            nc.scalar.activation(pt_s[:, j, :, :], psS[:, :],
                                                 AF.Exp, scale=SCALE)
                        else:
                            nc.scalar.activation(
                                ptd_s[:, r, :, q0:],
                                psS[:, :].rearrange("p (h q) -> p h q", h=2)[:, :, q0:],
                                AF.Exp, scale=SCALE)
                            for hh in (0, 1):
                                # staircase mask on the true diagonal block
                                nc.vector.tensor_tensor(
                                    out=ptd_s[:, r, hh, q0:q0 + P],
                                    in0=ptd_s[:, r, hh, q0:q0 + P],
                                    in1=dmask_s[:, :], op=ALU.mult)
                        # PE filler while ScalarE works through the exps
                        # (hold chunks back to cover the c tail; early chunks
                        # have little exp backlog, so keep more in reserve)
                        if j % 2 == 1 and (len(fill_q) > (7 - 2 * c) or c == 3):
                            pop_filler(1)
                        # 2-slot score lookahead
                        if j + 2 < nj:
                            slotS[j % 2] = scores_slot(c, pair, j + 2)
                        # A@V for this slot: [V|1]^T @ P^T per head
                        for hh in (0, 1):
                            h = 2 * pair + hh
                            if r < 0:
                                rhs = pt_s[:, j, hh, :]
                                out = psyt[hh][0:65, :]
                            else:
                                rhs = ptd_s[:, r, hh, P * r:]
                                out = psyt[hh][0:65, P * r:]
                            nc.tensor.matmul(
                                out, v_s[:, j, h, 0:65], rhs,
                                start=(j == 0), stop=(j == nj - 1))

                    # pair tail: copy y~^T to bf16, transpose per 128-q block,
                    # batched reciprocal of the row sums, normalize into y
                    for hh in (0, 1):
                        h = 2 * pair + hh
                        ytmp = yt_pool.tile([P, 512], dt.bfloat16, name="ytmp",
                                            tag="ytmp")
                        nc.vector.tensor_copy(out=ytmp[0:65, :],
                                              in_=psyt[hh][0:65, :])
                        ptr = av_pool.tile([P, 4, 66], dt.bfloat16, name="ptr",
                                           tag="av")
                        for qi_loc in range(4):
                            nc.tensor.transpose(
                                ptr[:, qi_loc, 0:65],
                                ytmp[0:65, P * qi_loc:P * (qi_loc + 1)],
                                ident_s[0:65, 0:65])
                        linv = sp.tile([P, 4], dt.float32, name="linv", tag="linv")
                        nc.vector.reciprocal(linv[:, :], ptr[:, :, 64:65])
                        for qi_loc in range(4):
                            nc.vector.tensor_scalar(
                                out=y_s[:, 4 * c + qi_loc, h, :],
                                in0=ptr[:, qi_loc, 0:64],
                                scalar1=linv[:, qi_loc:qi_loc + 1], scalar2=None,
                                op0=ALU.mult)
                    pop_filler(1)

                # ---- c tail: transpose y -> y^T and output projection ----
                emit_tail(c)
                pop_filler(len(fill_q))

    nc.compile()
    return nc


def _prep_inputs(x, w_attn, b_attn, w_proj, b_proj):
    """Host-side shard prep: per-core input dicts (core ci = b*2 + hg)."""
    x = np.asarray(x, dtype=np.float32)
    w_attn = np.asarray(w_attn, dtype=np.float32)
    b_attn = np.asarray(b_attn, dtype=np.float32)
    w_proj = np.asarray(w_proj, dtype=np.float32)
    b_proj = np.asarray(b_proj, dtype=np.float32)

    # diagonal staircase mask [tk, q]: valid iff q >= tk
    dmask = (np.arange(P)[None, :] >= np.arange(P)[:, None]).astype(BF16)
    ident = np.eye(P, dtype=BF16)

    in_maps = []
    for b in range(B):
        xT = np.ascontiguousarray(x[b].T).astype(BF16)       # [C, T]
        for hg in range(2):
            lo = hg * 512
            wqk_flat = np.concatenate(
                [w_attn[:, lo:lo + 512], w_attn[:, 1024 + lo:1024 + lo + 512]],
                axis=1)                                       # [C, 1024]
            # permute to [m, p, k*128]: m = 128-col output block, rows split
            # into k-chunks of 128 partitions
            wqk = np.ascontiguousarray(
                wqk_flat.reshape(KC, P, 8, P).transpose(2, 1, 0, 3)
                .reshape(8, P, KC * P)).astype(BF16)
            wv = w_attn[:, 2048 + lo:2048 + lo + 512].astype(BF16)
            wproj = w_proj[lo:lo + 512, :].astype(BF16)       # [512, C]
            bqk = np.stack(
                [b_attn[lo + P * m:lo + P * (m + 1)] for m in range(4)] +
                [b_attn[1024 + lo + P * m:1024 + lo + P * (m + 1)] for m in range(4)],
                axis=1).astype(np.float32)                    # [128, 8]
            bv = np.broadcast_to(b_attn[2048 + lo:2048 + lo + 512],
                                 (P, 512)).astype(np.float32)
            bp = b_proj if hg == 0 else np.zeros_like(b_proj)
            bproj = np.broadcast_to(bp, (P, C)).astype(np.float32)
            in_maps.append({
                "xT": xT, "wqk": wqk, "wv": wv, "wproj": wproj,
                "bqk": np.ascontiguousarray(bqk), "bv": np.ascontiguousarray(bv),
                "bproj": np.ascontiguousarray(bproj),
                "dmask": np.ascontiguousarray(dmask), "ident": ident,
            })
    return in_maps


def kernel(x, w_attn, b_attn, w_proj, b_proj):
    global LAST_RESULT
    from concourse.bass_utils import run_bass_kernel_spmd

    if "nc" not in _CACHE:
        _CACHE["nc"] = _build_program()
    nc = _CACHE["nc"]

    in_maps = _prep_inputs(x, w_attn, b_attn, w_proj, b_proj)
    res = run_bass_kernel_spmd(nc, in_maps, core_ids=list(range(8)))
    LAST_RESULT = res

    out = np.zeros((B, T, C), dtype=np.float32)
    for b in range(B):
        out[b] = (res.results[2 * b]["out"].astype(np.float32) +
                  res.results[2 * b + 1]["out"].astype(np.float32))
    return out


# revision 44
# speedup vs baseline: 1.0072x; 1.0072x over previous
"""Causal self-attention (B=4, T=2048, C=1024, 16 heads) on 8 trn2 NeuronCores.

Sharding: core (b, hg) handles batch b (4) x head-group hg (2 groups of 8 heads).
Each core computes QKV projection for its batch restricted to its 8 heads,
flash-style causal attention, and the output projection restricted to its
heads' rows of w_proj -> a partial [T, C] output. Host sums the two partials
per batch (tensor-parallel unshard) and concatenates batches.

v2 layout (all bf16 matmul inputs, fp32 PSUM accumulation):
  - q-chunk-outer loop (c = 512 queries), head-pair inner. Per (c, pair, slot):
    scores pair (two K=64 row-tiled matmuls run concurrently on the PE),
    ScalarE exp (one instruction per slot; diagonal slots combine both heads
    via a strided AP), then the A@V matmuls for that slot accumulate into the
    pair's two PSUM tiles -- a 3-stage PE/ScalarE pipeline with 2-deep PSUM
    pools and a 2-slot score lookahead.
  - V carries an appended ones-column so A@V (lhsT=[V|1], N=512) yields
    y~^T = [64 weighted-V | row-sum].  The [65,512] result is copied to bf16
    and PE-transposed in 128-q blocks into one [P,4,65] PSUM tile; a single
    batched reciprocal + 4 tensor_scalar ops produce normalized y.
  - y [t,ch] is PE-transposed to y^T per q-chunk and the output projection +
    output DMA run per chunk, keeping the PE warm (no transpose-only tail that
    would let the HAM clock gate re-throttle) and spreading the output DMA.
  - The next chunk's QKV projection matmuls are injected into the attention
    slot loops as PE filler while ScalarE works through the exps; the last
    chunk's K/Q projections are injected just-in-time one head-pair ahead.
  - Input DMAs split across the two HWDGE queues (Sync: x^T + w_v; Scalar:
    everything else) with a fine-grained first chunk so the first V-proj
    matmul issues as early as possible.
"""

import numpy as np
import ml_dtypes

B, T, C, H, D = 4, 2048, 1024, 16, 64
P = 128
TC = T // P          # 16 t-chunks of 128
KC = C // P          # 8 contraction chunks of 128
NPAIR = 4            # head pairs per core (8 local heads)
SCALE = 0.125        # 1/sqrt(64)

_CACHE = {}
LAST_RESULT = None   # BassKernelResults of the most recent run (for test.py)

BF16 = ml_dtypes.bfloat16


def _build_program():
    import concourse.tile as tile
    import concourse.mybir as mybir
    from concourse import bacc

    dt = mybir.dt
    AF = mybir.ActivationFunctionType
    ALU = mybir.AluOpType

    nc = bacc.Bacc("TRN2", target_bir_lowering=False, debug=False, num_devices=8)

    # ---- DRAM I/O ----
    xT_d = nc.dram_tensor("xT", [C, T], dt.bfloat16, kind="ExternalInput").ap()
    # wqk pre-permuted on host: [m, p, k*128] (m = 8 output 128-col blocks,
    # Q blocks 0-3 then K blocks 4-7; p = partition; k = contraction chunk)
    wqk_d = nc.dram_tensor("wqk", [8, P, 1024], dt.bfloat16, kind="ExternalInput").ap()
    wv_d = nc.dram_tensor("wv", [C, 512], dt.bfloat16, kind="ExternalInput").ap()
    wproj_d = nc.dram_tensor("wproj", [512, C], dt.bfloat16, kind="ExternalInput").ap()
    bqk_d = nc.dram_tensor("bqk", [P, 8], dt.float32, kind="ExternalInput").ap()
    bv_d = nc.dram_tensor("bv", [P, 512], dt.float32, kind="ExternalInput").ap()
    bproj_d = nc.dram_tensor("bproj", [P, C], dt.float32, kind="ExternalInput").ap()
    dmask_d = nc.dram_tensor("dmask", [P, P], dt.bfloat16, kind="ExternalInput").ap()
    ident_d = nc.dram_tensor("ident", [P, P], dt.bfloat16, kind="ExternalInput").ap()
    out_d = nc.dram_tensor("out", [T, C], dt.bfloat16, kind="ExternalOutput").ap()

    with tile.TileContext(nc) as tc:
        with (
            tc.tile_pool(name="const", bufs=1) as cp,
            tc.tile_pool(name="outp", bufs=4) as op_pool,
            tc.tile_pool(name="small", bufs=8) as sp,
            tc.tile_pool(name="ytmp", bufs=3) as yt_pool,
            tc.tile_pool(name="psS", bufs=2, space="PSUM") as psS_pool,
            tc.tile_pool(name="psav", bufs=2, space="PSUM") as av_pool,
            tc.tile_pool(name="psshr", bufs=2, space="PSUM") as shr_pool,
        ):
            # ---- static SBUF tensors ----
            xT_s = cp.tile([P, KC, T], dt.bfloat16, name="xT_s")
            wqk_s = cp.tile([P, 8, KC, 128], dt.bfloat16, name="wqk_s")
            wv_s = cp.tile([P, KC, 512], dt.bfloat16, name="wv_s")
            wproj_s = cp.tile([P, 4, C], dt.bfloat16, name="wproj_s")
            bqk_s = cp.tile([P, 8], dt.float32, name="bqk_s")
            bv_s = cp.tile([P, 512], dt.float32, name="bv_s")
            bproj_s = cp.tile([P, C], dt.float32, name="bproj_s")
            dmask_s = cp.tile([P, P], dt.bfloat16, name="dmask_s")
            ident_s = cp.tile([P, P], dt.bfloat16, name="ident_s")
            qt_s = cp.tile([P, NPAIR, T], dt.bfloat16, name="qt_s")
            kt_s = cp.tile([P, NPAIR, T], dt.bfloat16, name="kt_s")
            v_s = cp.tile([P, TC, 8, 66], dt.bfloat16, name="v_s")   # [t, tc, head, V|1]
            y_s = cp.tile([P, TC, 8, D], dt.bfloat16, name="y_s")    # y natural [q, head, d]
            yT_s = cp.tile([P, 2, 4, 512], dt.bfloat16, name="yT_s")  # y^T per chunk (x2)
            pt_s = cp.tile([P, 12, 2, 512], dt.bfloat16, name="pt_s")   # exp(S^T) off-diag
            ptd_s = cp.tile([P, 4, 2, 512], dt.bfloat16, name="ptd_s")  # diag slots

            warm_s = cp.tile([P, 512], dt.bfloat16, name="warm_s")
            # ones column of V~; zero the diag P^T buffer once (sub-diagonal
            # regions are never written by the partial exps, so zeros persist)
            nc.vector.memset(warm_s[:], 0.0)
            nc.vector.memset(v_s[:, :, :, 64:65], 1.0)
            nc.vector.memset(ptd_s[:], 0.0)

            # ---- input DMAs split across the two HWDGE queues ----
            xT_src = xT_d.rearrange("(o p) t -> p o t", p=P)
            wv_src = wv_d.rearrange("(o p) m -> p o m", p=P)
            # Sync queue: the big tensors, in exact consumption order (fine
            # first chunks so the first V-proj matmul can issue early; wqk
            # split by head-pair column ranges to match qk emission order)
            nc.sync.dma_start(wv_s[:, 0:1, :], wv_src[:, 0:1, :])
            nc.sync.dma_start(xT_s[:, :, 0:128], xT_src[:, :, 0:128])
            nc.sync.dma_start(wv_s[:, 1:2, :], wv_src[:, 1:2, :])
            nc.sync.dma_start(wv_s[:, 2:4, :], wv_src[:, 2:4, :])
            nc.sync.dma_start(wv_s[:, 4:6, :], wv_src[:, 4:6, :])
            nc.sync.dma_start(wv_s[:, 6:8, :], wv_src[:, 6:8, :])
            nc.sync.dma_start(xT_s[:, :, 128:256], xT_src[:, :, 128:256])
            nc.sync.dma_start(xT_s[:, :, 256:512], xT_src[:, :, 256:512])
            for p_ in range(4):
                nc.sync.dma_start(wqk_s[:, p_, :, :], wqk_d[p_])
                nc.sync.dma_start(wqk_s[:, 4 + p_, :, :], wqk_d[4 + p_])
            for q8 in range(2, 8):
                nc.sync.dma_start(xT_s[:, :, 256 * q8:256 * (q8 + 1)],
                                  xT_src[:, :, 256 * q8:256 * (q8 + 1)])
            # Scalar queue: small/late tensors (prologue-only; exps start later)
            nc.scalar.dma_start(bv_s[:], bv_d)
            nc.scalar.dma_start(bqk_s[:], bqk_d)
            nc.scalar.dma_start(dmask_s[:], dmask_d)
            nc.scalar.dma_start(ident_s[:], ident_d)
            nc.scalar.dma_start(wproj_s[:], wproj_d.rearrange("(o p) m -> p o m", p=P))
            nc.scalar.dma_start(bproj_s[:], bproj_d)

            # ---- projection chunk emitters (prologue + slot-loop fillers) ----
            def vproj_chunk(tcx):
                psv = shr_pool.tile([P, 512], dt.float32, name="psv", tag="shr")
                for k in range(KC):
                    nc.tensor.matmul(psv[:, :],
                                     xT_s[:, k, P * tcx:P * (tcx + 1)],
                                     wv_s[:, k, :],
                                     start=(k == 0), stop=(k == KC - 1))
                nc.vector.tensor_add(
                    out=v_s[:, tcx, :, 0:64],
                    in0=psv[:, :].rearrange("a (h d) -> a h d", h=8),
                    in1=bv_s[:, :].rearrange("a (h d) -> a h d", h=8),
                )

            def qkproj_chunk(m, t4):
                dst = qt_s if m < 4 else kt_s
                psq = shr_pool.tile([P, 512], dt.float32, name="psq", tag="shr")
                for k in range(KC):
                    nc.tensor.matmul(psq[:, :],
                                     wqk_s[:, m, k, :],
                                     xT_s[:, k, 512 * t4:512 * (t4 + 1)],
                                     start=(k == 0), stop=(k == KC - 1))
                nc.vector.tensor_scalar(
                    out=dst[:, m % 4, 512 * t4:512 * (t4 + 1)],
                    in0=psq[:, :], scalar1=bqk_s[:, m:m + 1], scalar2=None,
                    op0=ALU.add)

            from collections import deque
            fill_q = deque()

            def pop_filler(n=1):
                for _ in range(n):
                    if fill_q:
                        fill_q.popleft()()

            def stage2_chunk(c, tcx_loc, cc):
                """Transpose one [128q, 128ch] block of y into y^T for proj."""
                tcx = 4 * c + tcx_loc
                pstr = shr_pool.tile([P, P], dt.bfloat16, name="pstr", tag="shr")
                nc.tensor.transpose(pstr[:, :],
                                    y_s[:, tcx, 2 * cc:2 * cc + 2, :],
                                    ident_s[:, :])
                nc.vector.tensor_copy(
                    out=yT_s[:, c % 2, cc, P * tcx_loc:P * (tcx_loc + 1)],
                    in_=pstr[:, :])

            # ---- HAM warmup: dummy matmuls on the zeroed tile bridge the
            # clock-gate activity window across the head's DMA stalls so the
            # PE-dense QK-projection phase runs at full clock ----
            pswarm = shr_pool.tile([P, 512], dt.float32, name="pswarm", tag="shr")
            for wi in range(12):
                nc.tensor.matmul(pswarm[:, :], warm_s[:, 0:128], warm_s[:, :],
                                 start=(wi == 0), stop=(wi == 11))

            # ---- c=0 prologue (emitted directly). First 4 t-chunks run
            # k-outer so the PE consumes each wv k-chunk as it lands instead
            # of stalling on the full wv upfront ----
            psv0 = [shr_pool.tile([P, 512], dt.float32, name=f"psv0_{t}",
                                  tag="shr") for t in range(2)]
            psv1 = [psS_pool.tile([P, 512], dt.float32, name=f"psv1_{t}",
                                  tag="psS") for t in range(2)]
            pv = psv0 + psv1
            for k in range(KC):
                for tcx in range(4):
                    nc.tensor.matmul(pv[tcx][:, :],
                                     xT_s[:, k, P * tcx:P * (tcx + 1)],
                                     wv_s[:, k, :],
                                     start=(k == 0), stop=(k == KC - 1))
            for tcx in range(4):
                nc.vector.tensor_add(
                    out=v_s[:, tcx, :, 0:64],
                    in0=pv[tcx][:, :].rearrange("a (h d) -> a h d", h=8),
                    in1=bv_s[:, :].rearrange("a (h d) -> a h d", h=8),
                )
            for m in (0, 4, 1, 5, 2, 6, 3, 7):
                qkproj_chunk(m, 0)

            # ---- helper: scores pair for (c, pair, slot j) ----
            def scores_slot(c, pair, j):
                r = j - 4 * c
                q0 = P * r if r >= 0 else 0
                psS = psS_pool.tile([P, 1024], dt.float32, name="psS", tag="psS")
                for hh in (0, 1):
                    base = 64 * hh
                    nc.tensor.matmul(
                        psS[:, 512 * hh + q0:512 * (hh + 1)],
                        kt_s[base:base + 64, pair, P * j:P * (j + 1)],
                        qt_s[base:base + 64, pair, 512 * c + q0:512 * (c + 1)],
                        start=True, stop=True)
                return psS

            def emit_tail(c):
                """Chunk c's y -> y^T transposes and output projection."""
                cb = c % 2
                # cc outer: head-pairs 0-2 are ready before pair 3's normalize
                # chain finishes, so their transposes overlap it; pair 3 last
                for cc in range(4):
                    for tcx_loc in range(4):
                        stage2_chunk(c, tcx_loc, cc)
                    pop_filler(1)
                for tcx_loc in range(4):
                    tcx = 4 * c + tcx_loc
                    for co in range(2):
                        psp = psS_pool.tile([P, 512], dt.float32, name="psp",
                                            tag="psS")
                        for cc in range(4):
                            nc.tensor.matmul(
                                psp[:, :],
                                yT_s[:, cb, cc, P * tcx_loc:P * (tcx_loc + 1)],
                                wproj_s[:, cc, 512 * co:512 * (co + 1)],
                                start=(cc == 0), stop=(cc == 3))
                        ot = op_pool.tile([P, 512], dt.bfloat16, name="ot",
                                          tag="ot")
                        nc.vector.tensor_add(out=ot[:, :], in0=psp[:, :],
                                             in1=bproj_s[:, 512 * co:512 * (co + 1)])
                        nc.sync.dma_start(
                            out_d[P * tcx:P * (tcx + 1), 512 * co:512 * (co + 1)],
                            ot[:, :])
                    pop_filler(1)

            # ---- main attention loop: q-chunk outer, head-pair inner ----
            for c in range(4):
                nj = 4 * c + 4
                # stock the filler queue with next chunk's projections
                if c == 0:
                    for tcx in range(4, 8):
                        fill_q.append(lambda tcx=tcx: vproj_chunk(tcx))
                    for m in (0, 4, 1, 5, 2, 6, 3, 7):
                        fill_q.append(lambda m=m: qkproj_chunk(m, 1))
                elif c == 1:
                    for tcx in range(8, 12):
                        fill_q.append(lambda tcx=tcx: vproj_chunk(tcx))
                    for m in (0, 4, 1, 5, 2, 6, 3, 7):
                        fill_q.append(lambda m=m: qkproj_chunk(m, 2))
                elif c == 2:
                    for tcx in range(12, 16):
                        fill_q.append(lambda tcx=tcx: vproj_chunk(tcx))
                    for m in (0, 4):
                        fill_q.append(lambda m=m: qkproj_chunk(m, 3))

                for pair in range(NPAIR):
                    if c == 3 and pair < 3:
                        # just-in-time Q/K projections for the next head pair
                        for m in (pair + 1, 5 + pair):
                            fill_q.append(lambda m=m: qkproj_chunk(m, 3))


                    psyt = [av_pool.tile([P, 512], dt.float32, name=f"psyt{hh}",
                                         tag="av") for hh in (0, 1)]
                    slotS = [scores_slot(c, pair, 0), scores_slot(c, pair, 1)
                             if nj > 1 else None]
                    for j in range(nj):
                        r = j - 4 * c
                        q0 = P * r if r >= 0 else 0
                        psS = slotS[j % 2]
                        # exp( S^T * scale ), fp32 psum -> bf16 sbuf
                        if r < 0:
                            nc.scalar.activation(pt_s[:, j, :, :], psS[:, :],
                                                 AF.Exp, scale=SCALE)
                        else:
                            nc.scalar.activation(
                                ptd_s[:, r, :, q0:],
                                psS[:, :].rearrange("p (h q) -> p h q", h=2)[:, :, q0:],
                                AF.Exp, scale=SCALE)
                            for hh in (0, 1):
                                # staircase mask on the true diagonal block
                                nc.vector.tensor_tensor(
                                    out=ptd_s[:, r, hh, q0:q0 + P],
                                    in0=ptd_s[:, r, hh, q0:q0 + P],
                                    in1=dmask_s[:, :], op=ALU.mult)
                        # PE filler while ScalarE works through the exps
                        # (hold a few chunks back to cover the c tail)
                        if j % 2 == 1 and (len(fill_q) > 4 or c == 3):
                            pop_filler(1)
                        # 2-slot score lookahead
                        if j + 2 < nj:
                            slotS[j % 2] = scores_slot(c, pair, j + 2)
                        # A@V for this slot: [V|1]^T @ P^T per head
                        for hh in (0, 1):
                            h = 2 * pair + hh
                            if r < 0:
                                rhs = pt_s[:, j, hh, :]
                                out = psyt[hh][0:65, :]
                            else:
                                rhs = ptd_s[:, r, hh, P * r:]
                                out = psyt[hh][0:65, P * r:]
                            nc.tensor.matmul(
                                out, v_s[:, j, h, 0:65], rhs,
                                start=(j == 0), stop=(j == nj - 1))

                    # pair tail: copy y~^T to bf16, transpose per 128-q block,
                    # batched reciprocal of the row sums, normalize into y
                    for hh in (0, 1):
                        h = 2 * pair + hh
                        ytmp = yt_pool.tile([P, 512], dt.bfloat16, name="ytmp",
                                            tag="ytmp")
                        nc.vector.tensor_copy(out=ytmp[0:65, :],
                                              in_=psyt[hh][0:65, :])
                        ptr = av_pool.tile([P, 4, 66], dt.bfloat16, name="ptr",
                                           tag="av")
                        for qi_loc in range(4):
                            nc.tensor.transpose(
                                ptr[:, qi_loc, 0:65],
                                ytmp[0:65, P * qi_loc:P * (qi_loc + 1)],
                                ident_s[0:65, 0:65])
                        linv = sp.tile([P, 4], dt.float32, name="linv", tag="linv")
                        nc.vector.reciprocal(linv[:, :], ptr[:, :, 64:65])
                        for qi_loc in range(4):
                            nc.vector.tensor_scalar(
                                out=y_s[:, 4 * c + qi_loc, h, :],
                                in0=ptr[:, qi_loc, 0:64],
                                scalar1=linv[:, qi_loc:qi_loc + 1], scalar2=None,
                                op0=ALU.mult)
                    pop_filler(1)

                # ---- c tail: transpose y -> y^T and output projection ----
                emit_tail(c)
                pop_filler(len(fill_q))

    nc.compile()
    return nc


def _prep_inputs(x, w_attn, b_attn, w_proj, b_proj):
    """Host-side shard prep: per-core input dicts (core ci = b*2 + hg)."""
    x = np.asarray(x, dtype=np.float32)
    w_attn = np.asarray(w_attn, dtype=np.float32)
    b_attn = np.asarray(b_attn, dtype=np.float32)
    w_proj = np.asarray(w_proj, dtype=np.float32)
    b_proj = np.asarray(b_proj, dtype=np.float32)

    # diagonal staircase mask [tk, q]: valid iff q >= tk
    dmask = (np.arange(P)[None, :] >= np.arange(P)[:, None]).astype(BF16)
    ident = np.eye(P, dtype=BF16)

    in_maps = []
    for b in range(B):
        xT = np.ascontiguousarray(x[b].T).astype(BF16)       # [C, T]
        for hg in range(2):
            lo = hg * 512
            wqk_flat = np.concatenate(
                [w_attn[:, lo:lo + 512], w_attn[:, 1024 + lo:1024 + lo + 512]],
                axis=1)                                       # [C, 1024]
            # permute to [m, p, k*128]: m = 128-col output block, rows split
            # into k-chunks of 128 partitions
            wqk = np.ascontiguousarray(
                wqk_flat.reshape(KC, P, 8, P).transpose(2, 1, 0, 3)
                .reshape(8, P, KC * P)).astype(BF16)
            wv = w_attn[:, 2048 + lo:2048 + lo + 512].astype(BF16)
            wproj = w_proj[lo:lo + 512, :].astype(BF16)       # [512, C]
            bqk = np.stack(
                [b_attn[lo + P * m:lo + P * (m + 1)] for m in range(4)] +
                [b_attn[1024 + lo + P * m:1024 + lo + P * (m + 1)] for m in range(4)],
                axis=1).astype(np.float32)                    # [128, 8]
            bv = np.broadcast_to(b_attn[2048 + lo:2048 + lo + 512],
                                 (P, 512)).astype(np.float32)
            bp = b_proj if hg == 0 else np.zeros_like(b_proj)
            bproj = np.broadcast_to(bp, (P, C)).astype(np.float32)
            in_maps.append({
                "xT": xT, "wqk": wqk, "wv": wv, "wproj": wproj,
                "bqk": np.ascontiguousarray(bqk), "bv": np.ascontiguousarray(bv),
                "bproj": np.ascontiguousarray(bproj),
                "dmask": np.ascontiguousarray(dmask), "ident": ident,
            })
    return in_maps


def kernel(x, w_attn, b_attn, w_proj, b_proj):
    global LAST_RESULT
    from concourse.bass_utils import run_bass_kernel_spmd

    if "nc" not in _CACHE:
        _CACHE["nc"] = _build_program()
    nc = _CACHE["nc"]

    in_maps = _prep_inputs(x, w_attn, b_attn, w_proj, b_proj)
    res = run_bass_kernel_spmd(nc, in_maps, core_ids=list(range(8)))
    LAST_RESULT = res

    out = np.zeros((B, T, C), dtype=np.float32)
    for b in range(B):
        out[b] = (res.results[2 * b]["out"].astype(np.float32) +
                  res.results[2 * b + 1]["out"].astype(np.float32))
    return out
